# revision 27
# baseline (speedup 1.0000x reference)
"""DiscGCN (3-layer GCN, 100k nodes / 1.6M edges) on 8 Trainium2 NeuronCores.

Strategy: nodes + incident (dst) edges sharded across the 8 cores; per layer
the transformed features (pre-scaled by dinv[src]) are AllGathered (in
int16-addressable chunks) into a replicated DRAM table, per-edge messages are
fetched with dma_gather round-robined over 4 SWDGE queues, and aggregated per
256-wide destination super-strip with one-hot matmuls accumulating in PSUM
(f32). The dst-side dinv factor is applied once per super-strip after
accumulation. Pad slots are masked by dstp=-1 (zero row in the one-hot A
matrix). Tables/messages bf16; accumulation f32.
"""
import numpy as np

import concourse.bacc as bacc
import concourse.bass as bass
import concourse.tile as tile
from concourse import mybir
from concourse.bass_utils import run_bass_kernel_spmd
from concourse.masks import make_identity

N_CORES = 8
D = 128
P = 128

N_NODES = 100000
NLOC = 12500

N_QUEUES = 4


def build(ei, n_nodes, nloc, batch_groups=64, sw=2):
    nlocp = ((nloc + P - 1) // P) * P
    n_strips = nlocp // P
    swf = sw * P
    n_super = (n_strips + sw - 1) // sw
    vtab = N_CORES * nlocp
    n_chunks = max(1, int(np.ceil(vtab / 32768 + 1e-9)))
    sl = int(np.ceil(nlocp / n_chunks / P)) * P
    while N_CORES * sl > 32767:
        n_chunks += 1
        sl = int(np.ceil(nlocp / n_chunks / P)) * P
    chunk_off = [min(j * sl, nlocp) for j in range(n_chunks + 1)]
    chunk_len = [chunk_off[j + 1] - chunk_off[j] for j in range(n_chunks)]
    coff = np.array(chunk_off[:-1])

    src = np.asarray(ei[0], dtype=np.int64)
    dst = np.asarray(ei[1], dtype=np.int64)

    deg = 1.0 + np.bincount(dst, minlength=n_nodes).astype(np.float32)
    dinv = (1.0 / np.sqrt(deg)).astype(np.float32)

    src_core = src // nloc
    src_loc = src % nloc
    src_chunk = np.minimum(src_loc // sl, n_chunks - 1)
    src_idx = (src_core * np.array(chunk_len)[src_chunk]
               + (src_loc - coff[src_chunk]))
    src_rowabs = N_CORES * coff[src_chunk] + src_idx

    core = dst // nloc
    dloc = dst % nloc
    sstrip = dloc // swf
    dstp = dloc % swf

    per_core = []
    for c in range(N_CORES):
        m = core == c
        e_idx = src_idx[m]
        e_rowabs = src_rowabs[m]
        e_ss = sstrip[m]
        e_dstp = dstp[m]
        e_chunk = src_chunk[m]
        order = np.lexsort((e_idx, e_ss, e_chunk))
        per_core.append((e_chunk[order], e_ss[order], e_idx[order],
                         e_rowabs[order], e_dstp[order]))

    run_groups = np.zeros((n_chunks, n_super), np.int64)
    run_lens = np.zeros((N_CORES, n_chunks, n_super), np.int64)
    for c in range(N_CORES):
        e_chunk, e_ss = per_core[c][0], per_core[c][1]
        key = e_chunk * n_super + e_ss
        cnt = np.bincount(key, minlength=n_chunks * n_super).reshape(
            n_chunks, n_super)
        run_lens[c] = cnt
        run_groups = np.maximum(run_groups, (cnt + P - 1) // P)

    # self groups first: groups [0, n_strips), padded to batch boundary
    self_off = ((n_strips + batch_groups - 1) // batch_groups) * batch_groups
    grid = []  # (chunk, sstrip, group_start, n_groups)
    gstart = self_off
    for ck in range(n_chunks):
        for ss in range(n_super):
            ng = int(run_groups[ck, ss])
            if ng == 0:
                continue
            grid.append((ck, ss, gstart, ng))
            gstart += ng
    n_groups_reg = gstart
    n_groups_reg_pad = ((n_groups_reg + batch_groups - 1)
                        // batch_groups) * batch_groups
    n_batches = n_groups_reg_pad // batch_groups
    n_groups_pad = n_groups_reg_pad
    nslot = n_groups_pad * P

    idx16 = np.full((N_CORES, nslot), -1, np.int16)
    slot_dstp = np.full((N_CORES, nslot), -1.0, np.float32)
    slot_row_abs = np.zeros((N_CORES, nslot), np.int64)

    group_chunk = np.zeros(n_groups_pad, np.int64)
    group_ss = np.full(n_groups_pad, -1, np.int64)
    for (ck, ss, g0, ng) in grid:
        group_chunk[g0:g0 + ng] = ck
        group_ss[g0:g0 + ng] = ss
    if grid and n_groups_reg < n_groups_reg_pad:
        group_chunk[n_groups_reg:n_groups_reg_pad] = grid[-1][0]

    base_of_chunk = N_CORES * coff
    for c in range(N_CORES):
        e_chunk, e_ss, e_idx, e_rowabs, e_dstp = per_core[c]
        pos = 0
        for (ck, ss, g0, ng) in grid:
            ln = int(run_lens[c, ck, ss])
            sl0 = g0 * P
            idx16[c, sl0:sl0 + ln] = e_idx[pos:pos + ln].astype(np.int16)
            slot_row_abs[c, sl0:sl0 + ln] = e_rowabs[pos:pos + ln]
            slot_dstp[c, sl0:sl0 + ln] = e_dstp[pos:pos + ln].astype(np.float32)
            pos += ln
        assert pos == len(e_idx)
        # pad slots of regular groups gather row 0 of their chunk (valid,
        # finite data); dstp stays -1 so the one-hot A row is all zero.
        rng = np.arange(self_off * P, n_groups_reg_pad * P)
        mpad = idx16[c, self_off * P:n_groups_reg_pad * P] < 0
        gch = group_chunk[rng // P]
        idx16[c, self_off * P:n_groups_reg_pad * P][mpad] = 0
        slot_row_abs[c, self_off * P:n_groups_reg_pad * P][mpad] = \
            base_of_chunk[gch][mpad]

    # self groups: one group per strip; slot p -> local node s*P+p.
    # Table rows are pre-scaled by dinv[src]; output is scaled by dinv[dst];
    # so a plain one-hot diagonal gives the dinv^2 self contribution.
    self_groups = []  # (batch, g_rel, strip)
    for s in range(n_strips):
        g_abs = s
        b = g_abs // batch_groups
        self_groups.append((b, g_abs - b * batch_groups, s))
        group_ss[g_abs] = s // sw
        sl0 = g_abs * P
        for c in range(N_CORES):
            pvec = np.arange(P)
            dlocv = s * P + pvec
            valid = dlocv < nloc
            dp = np.full(P, -1.0, np.float32)
            dp[valid] = ((s % sw) * P + pvec[valid]).astype(np.float32)
            slot_dstp[c, sl0:sl0 + P] = dp
            jv = np.minimum(dlocv // sl, n_chunks - 1)
            slot_row_abs[c, sl0:sl0 + P] = (
                N_CORES * coff[jv] + c * np.array(chunk_len)[jv]
                + (dlocv - coff[jv]))  # for simulation

    # gather segments: per batch, consecutive regular groups sharing a chunk,
    # <=32 groups; (batch, glo, ghi, chunk, seg_id)
    segments = []
    for b in range(n_batches):
        glo = b * batch_groups
        ghi_b = n_groups_reg_pad if b == n_batches - 1 else (b + 1) * batch_groups
        if glo < self_off:
            glo = self_off  # skip self batches
            if glo >= ghi_b:
                continue
        g = glo
        while g < ghi_b:
            ck = group_chunk[g]
            h = g
            while h < ghi_b and group_chunk[h] == ck and h - g < 32:
                h += 1
            sid = len(segments)
            segments.append((b, g - b * batch_groups, h - b * batch_groups,
                             int(ck), sid))
            g = h

    # subruns: self first, then regular (grid order); each subrun is a
    # contiguous group range within one batch targeting one super-strip.
    subruns = []  # (batch, ss, glo, ghi, first, last)
    seen_first = set()
    for (b, g_rel, s) in self_groups:
        ss = s // sw
        subruns.append([b, ss, g_rel, g_rel + 1, ss not in seen_first, False])
        seen_first.add(ss)
    for (ck, ss, g0, ng) in grid:
        g = g0
        while g < g0 + ng:
            b = g // batch_groups
            ghi = min(g0 + ng, (b + 1) * batch_groups)
            subruns.append([b, ss, g - b * batch_groups, ghi - b * batch_groups,
                            ss not in seen_first, False])
            seen_first.add(ss)
            g = ghi
    # mark the last subrun of each super-strip (epilogue point)
    last_of = {}
    for i, sr in enumerate(subruns):
        last_of[sr[1]] = i
    for i in last_of.values():
        subruns[i][5] = True
    subruns = [tuple(sr) for sr in subruns]

    bg16 = batch_groups * P // 16
    idx_wrapped = np.zeros((N_CORES, 128, n_batches * bg16), np.int16)
    for c in range(N_CORES):
        for b in range(n_batches):
            fl = idx16[c, b * batch_groups * P:(b + 1) * batch_groups * P]
            w = fl.reshape(-1, 16).T
            idx_wrapped[c, :, b * bg16:(b + 1) * bg16] = np.tile(w, (8, 1))

    dstp_g = np.zeros((N_CORES, 128, n_groups_pad), np.float32)
    for c in range(N_CORES):
        dstp_g[c] = slot_dstp[c].reshape(n_groups_pad, P).T

    # per-core dinv in slab layout [P, n_strips] and row layout [1, nlocp]
    dinvT = np.zeros((N_CORES, P, n_strips), np.float32)
    dinv_row = np.zeros((N_CORES, 1, nlocp), np.float32)
    for c in range(N_CORES):
        dl = np.zeros(nlocp, np.float32)
        r = dinv[c * nloc:(c + 1) * nloc]
        dl[:r.shape[0]] = r
        dinvT[c] = dl.reshape(n_strips, P).T
        dinv_row[c, 0] = dl

    return dict(
        nlocp=nlocp, n_strips=n_strips, n_super=n_super, sw=sw, swf=swf,
        vtab=vtab, n_chunks=n_chunks, chunk_off=chunk_off, chunk_len=chunk_len,
        n_batches=n_batches, batch_groups=batch_groups, nslot=nslot,
        n_groups_pad=n_groups_pad, grid=grid, group_ss=group_ss,
        segments=segments, subruns=subruns, self_groups=self_groups,
        idx_wrapped=idx_wrapped, dstp_g=dstp_g,
        slot_row_abs=slot_row_abs, slot_dstp=slot_dstp,
        dinv=dinv, dinvT=dinvT, dinv_row=dinv_row,
    )


def host_simulate(ei, x, Ws, bs, meta, nloc):
    """Vectorized numpy simulation of the device algorithm (layout check)."""
    nlocp, vtab, swf = meta["nlocp"], meta["vtab"], meta["swf"]
    group_ss = meta["group_ss"]
    slot_ss = np.repeat(group_ss, P)
    dinv = meta["dinv"]
    chunk_off, chunk_len = meta["chunk_off"], meta["chunk_len"]

    def to_table(h_list):
        """chunk-interleaved layout: chunk j holds rows
        [8*off_j + c*len_j + (loc-off_j)] for core c, local row loc."""
        t = np.zeros((vtab, h_list[0].shape[1]), np.float32)
        for j in range(len(chunk_len)):
            o, ln = chunk_off[j], chunk_len[j]
            for c in range(N_CORES):
                base = N_CORES * o + c * ln
                t[base:base + ln] = h_list[c][o:o + ln]
        return t

    x_loc = []
    dinv_loc = []
    for c in range(N_CORES):
        xx = np.zeros((nlocp, x.shape[1]), np.float32)
        r = x[c * nloc:(c + 1) * nloc]
        xx[:r.shape[0]] = r
        x_loc.append(xx)
        dl = np.zeros((nlocp,), np.float32)
        dl[:r.shape[0]] = dinv[c * nloc:c * nloc + r.shape[0]]
        dinv_loc.append(dl)

    h = x_loc
    outs = []
    for l, (W, b) in enumerate(zip(Ws, bs)):
        last = l == len(Ws) - 1
        if not last:
            t = to_table([dinv_loc[c][:, None] * (h[c] @ W)
                          for c in range(N_CORES)])
        else:
            t = to_table([dinv_loc[c][:, None] * h[c] for c in range(N_CORES)])
        hn = []
        for c in range(N_CORES):
            rows = meta["slot_row_abs"][c]
            msgs = t[rows]
            dd = meta["slot_dstp"][c]
            valid = (dd >= 0) & (slot_ss >= 0)
            a = np.zeros((nlocp, t.shape[1]), np.float32)
            tgt = slot_ss * swf + dd.astype(np.int64)
            np.add.at(a, tgt[valid], msgs[valid])
            a = a * dinv_loc[c][:, None]
            if not last:
                hn.append(np.maximum(a + b[None, :], 0.0))
            else:
                outs.append(1.0 / (1.0 + np.exp(-(a @ W + b)))[:nloc])
        if not last:
            h = hn
    return np.concatenate(outs, axis=0)


f32 = mybir.dt.float32
bf16 = mybir.dt.bfloat16
f16 = mybir.dt.float16
i16 = mybir.dt.int16
AF = mybir.ActivationFunctionType
ALU = mybir.AluOpType


def build_program(meta, use_cc=True, mdt=bf16):
    """meta: from build(). Returns finalized nc."""
    nlocp = meta["nlocp"]
    n_strips = meta["n_strips"]
    vtab = meta["vtab"]
    chunk_off = meta["chunk_off"]
    chunk_len = meta["chunk_len"]
    nb = meta["n_batches"]
    bg = meta["batch_groups"]
    sw = meta["sw"]
    swf = meta["swf"]
    n_super = meta["n_super"]
    assert swf <= 512

    segments = meta["segments"]          # (batch, glo, ghi, chunk, seg_id)
    self_groups = meta["self_groups"]    # (batch, g_rel, strip)
    subruns = meta["subruns"]            # (batch, ss, glo, ghi, first, last)
    bg16 = bg * P // 16

    nc = bacc.Bacc(None, target_bir_lowering=False, num_swdge_queues=N_QUEUES)

    # ---- I/O ----
    xT = nc.dram_tensor("xT", [P, nlocp], f32, kind="ExternalInput")
    W1 = nc.dram_tensor("W1", [P, D], f32, kind="ExternalInput")
    W2 = nc.dram_tensor("W2", [P, D], f32, kind="ExternalInput")
    W3 = nc.dram_tensor("W3", [P, 1], f32, kind="ExternalInput")
    b1 = nc.dram_tensor("b1", [P, 1], f32, kind="ExternalInput")
    b2 = nc.dram_tensor("b2", [P, 1], f32, kind="ExternalInput")
    b3 = nc.dram_tensor("b3", [P, 1], f32, kind="ExternalInput")
    iota = nc.dram_tensor("iota", [P, swf], f16, kind="ExternalInput")
    idxd = nc.dram_tensor("idxd", [P, nb * bg16], i16, kind="ExternalInput")
    dstpd = nc.dram_tensor("dstpd", [P, nb * bg], f16, kind="ExternalInput")
    dinvTd = nc.dram_tensor("dinvTd", [P, n_strips], f32, kind="ExternalInput")
    dinvRd = nc.dram_tensor("dinvRd", [P, nlocp], f32, kind="ExternalInput")
    out = nc.dram_tensor("out", [nlocp, 1], f32, kind="ExternalOutput")

    # ---- internal DRAM ----
    n_sl = len(chunk_len)
    bounce = [[nc.dram_tensor(f"bounce{l}_{j}", [chunk_len[j], D], mdt)
               for j in range(n_sl)] for l in range(3)]

    _co = {}

    def chunk_off_of(bt):
        return _co[bt.name]

    def bounce_rows(l, r0, r1):
        """list of (tensor, lo, hi) covering local rows [r0, r1)."""
        parts = []
        for j in range(n_sl):
            o = chunk_off[j]
            lo = max(r0, o)
            hi = min(r1, o + chunk_len[j])
            if lo < hi:
                parts.append((bounce[l][j], lo - o, hi - o))
        return parts
    for l in range(3):
        for j in range(n_sl):
            _co[bounce[l][j].name] = chunk_off[j]
    tbl = [nc.dram_tensor(f"tbl{l}", [vtab, D], mdt,
                          addr_space="Shared" if use_cc else "Local")
           for l in range(3)]

    with tile.TileContext(nc) as tc:
        with (
            tc.tile_pool(name="const", bufs=1) as cpool,
            tc.tile_pool(name="slab", bufs=1) as slab_pool,
            tc.tile_pool(name="stream", bufs=5) as spool,
            tc.tile_pool(name="abuf", bufs=4) as apool,
            tc.tile_pool(name="msg", bufs=8) as mpool,
            tc.tile_pool(name="stage", bufs=2) as stpool,
            tc.tile_pool(name="accp", bufs=4, space="PSUM") as acc_pool,
            tc.tile_pool(name="gp", bufs=2, space="PSUM") as gp_pool,
        ):
            # constants
            iota_t = cpool.tile([P, swf], f16)
            nc.sync.dma_start(out=iota_t[:], in_=iota[:])
            w_t = []
            for l, W in enumerate((W1, W2)):
                t = cpool.tile([P, D], f32, tag=f"w{l}")
                nc.sync.dma_start(out=t[:], in_=W[:])
                w_t.append(t)
            w3_t = cpool.tile([P, 1], f32)
            nc.sync.dma_start(out=w3_t[:], in_=W3[:])
            b_t = []
            for l, B in enumerate((b1, b2)):
                t = cpool.tile([P, 1], f32, tag=f"b{l}")
                nc.sync.dma_start(out=t[:], in_=B[:])
                b_t.append(t)
            b3_t = cpool.tile([P, 1], f32)
            nc.sync.dma_start(out=b3_t[:], in_=b3[:])
            dinvT_t = cpool.tile([P, n_strips], f32)
            nc.sync.dma_start(out=dinvT_t[:], in_=dinvTd[:])
            ident = cpool.tile([P, P], f32)
            make_identity(nc, ident[:])

            for _i in range(8):
                zm = mpool.tile([P, bg, D], mdt, tag="msg")
                nc.vector.memset(zm[:], 0.0)
            slab0 = slab_pool.tile([P, nlocp], f32, tag="s0")

            xt_sb = slab_pool.tile([P, nlocp], f32, tag="s0")
            nc.sync.dma_start(out=xt_sb[:], in_=xT[:])

            def produce_g(l):
                """round l in {0,1}: rows of dinv * (h @ W_l) -> bounce[l]."""
                stage = None
                for s in range(n_strips):
                    if l == 0:
                        lhsT = xt_sb[:, s * P:(s + 1) * P]
                    else:
                        lhsT = slab0[:, s * P:(s + 1) * P]
                    ps = gp_pool.tile([P, D], f32, space="PSUM", tag="gps")
                    nc.tensor.matmul(ps[:], lhsT, w_t[l][:], start=True, stop=True)
                    k = s % 4
                    if k == 0:
                        stage = stpool.tile([P, 4, D], mdt, tag="gstage")
                    nc.scalar.activation(out=stage[:, k, :], in_=ps[:],
                                         func=AF.Copy,
                                         scale=dinvT_t[:, s:s + 1])
                    if k == 3 or s == n_strips - 1:
                        s0 = s - k
                        for (bt, lo, hi) in bounce_rows(l, s0 * P, (s + 1) * P):
                            dv = bt[lo:hi, :].rearrange("(g p) f -> p g f", p=P)
                            g0 = (chunk_off_of(bt) + lo - s0 * P) // P
                            nc.sync.dma_start(
                                out=dv, in_=stage[:, g0:g0 + (hi - lo) // P, :])

            def produce_h2_rows():
                """transpose slab ([f, n]) into bounce[2] rows, x dinv."""
                stage = None
                for s in range(n_strips):
                    ps = gp_pool.tile([P, P], f32, space="PSUM", tag="gps")
                    nc.tensor.transpose(
                        out=ps[:], in_=slab0[:, s * P:(s + 1) * P],
                        identity=ident[:])
                    k = s % 4
                    if k == 0:
                        stage = stpool.tile([P, 4, D], mdt, tag="gstage")
                    nc.scalar.activation(out=stage[:, k, :], in_=ps[:],
                                         func=AF.Copy,
                                         scale=dinvT_t[:, s:s + 1])
                    if k == 3 or s == n_strips - 1:
                        s0 = s - k
                        for (bt, lo, hi) in bounce_rows(2, s0 * P, (s + 1) * P):
                            dv = bt[lo:hi, :].rearrange("(g p) f -> p g f", p=P)
                            g0 = (chunk_off_of(bt) + lo - s0 * P) // P
                            nc.sync.dma_start(
                                out=dv, in_=stage[:, g0:g0 + (hi - lo) // P, :])

            seg_counter = [0]

            def message_passing(l, relu_bias):
                """aggregate tbl[l] messages into slab0 (T layout)."""
                t = tbl[l]
                msg_of_batch = {}
                dst_of_batch = {}

                def ensure_batch(b):
                    if b in msg_of_batch:
                        return
                    mt = mpool.tile([P, bg, D], mdt, tag="msg")
                    msg_of_batch[b] = mt
                    it = spool.tile([P, bg16], i16, tag="idx")
                    nc.sync.dma_start(out=it[:],
                                      in_=idxd[:, b * bg16:(b + 1) * bg16])
                    dt_ = spool.tile([P, bg], f16, tag="dst")
                    nc.sync.dma_start(out=dt_[:],
                                      in_=dstpd[:, b * bg:(b + 1) * bg])
                    dst_of_batch[b] = dt_
                    # emit gathers for this batch (round-robin SWDGE queues)
                    for (bb, glo, ghi, ck, sid) in segments:
                        if bb != b:
                            continue
                        n_idx = (ghi - glo) * P
                        o8 = N_CORES * chunk_off[ck]
                        l8 = N_CORES * chunk_len[ck]
                        nc.gpsimd.dma_gather(
                            mt[:, glo:ghi, :],
                            t[o8:o8 + l8, :],
                            it[:, glo * 8:ghi * 8],
                            n_idx, n_idx, D, single_packet=False,
                            queue_num=seg_counter[0] % N_QUEUES,
                        )
                        seg_counter[0] += 1
                    # self groups: DMA pre-scaled rows from bounce
                    runs = []
                    for (bb, g_rel, s) in self_groups:
                        if bb != b:
                            continue
                        if runs and runs[-1][1] == g_rel and runs[-1][3] == s:
                            runs[-1] = (runs[-1][0], g_rel + 1, runs[-1][2], s + 1)
                        else:
                            runs.append((g_rel, g_rel + 1, s, s + 1))
                    for (g0, g1, s0, s1) in runs:
                        for (bt, lo, hi) in bounce_rows(l, s0 * P, s1 * P):
                            dv = bt[lo:hi, :].rearrange("(g p) f -> p g f", p=P)
                            gg0 = g0 + (chunk_off_of(bt) + lo - s0 * P) // P
                            nc.sync.dma_start(
                                out=mt[:, gg0:gg0 + (hi - lo) // P, :], in_=dv)

                AB = 8  # groups per batched A-build
                a_of_batch = {}

                def ensure_abuilds(b):
                    if b in a_of_batch:
                        return
                    dt_ = dst_of_batch[b]
                    tiles = []
                    for w in range(bg // AB):
                        At = apool.tile([P, AB, swf], mdt, tag="A")
                        iota_b = bass.AP(iota_t[:].tensor, iota_t[:].offset,
                                         [iota_t[:].ap[0], [0, AB],
                                          iota_t[:].ap[1]])
                        nc.vector.tensor_tensor(
                            out=At[:], in0=iota_b,
                            in1=dt_[:, w * AB:(w + 1) * AB].to_broadcast(
                                [P, AB, swf]),
                            op=ALU.is_equal,
                        )
                        tiles.append(At)
                    a_of_batch[b] = tiles

                # subruns in order; accumulate in SBUF slab per super-strip
                for (b, ss, glo, ghi, first, last) in subruns:
                    ensure_batch(b)
                    ensure_abuilds(b)
                    mt = msg_of_batch[b]
                    at = a_of_batch[b]
                    wid = min(swf, nlocp - ss * swf)
                    ps = acc_pool.tile([P, swf], f32, space="PSUM", tag="acc")
                    for g in range(glo, ghi):
                        nc.tensor.matmul(
                            ps[:], mt[:, g, :], at[g // AB][:, g % AB, :],
                            start=(g == glo), stop=(g == ghi - 1),
                        )
                    dstv = slab0[:, ss * swf:ss * swf + wid]
                    if first:
                        nc.vector.tensor_copy(out=dstv, in_=ps[:, :wid])
                    else:
                        nc.vector.tensor_add(out=dstv, in0=dstv, in1=ps[:, :wid])
                    if last:
                        # dst-side dinv scale, then ReLU+bias (layers 1-2)
                        dbc = spool.tile([P, swf], f32, tag="dbc")
                        nc.sync.dma_start(
                            out=dbc[:, :wid],
                            in_=dinvRd[:, ss * swf:ss * swf + wid])
                        nc.vector.tensor_tensor(out=dstv, in0=dstv,
                                                in1=dbc[:, :wid],
                                                op=ALU.mult)
                        if relu_bias is not None:
                            nc.scalar.activation(out=dstv, in_=dstv,
                                                 func=AF.Relu,
                                                 bias=relu_bias[:])

            def allgather(l):
                if use_cc:
                    for _j in range(len(chunk_len)):
                        _o, _ln = chunk_off[_j], chunk_len[_j]
                        nc.gpsimd.collective_compute(
                            "AllGather", ALU.bypass,
                            ins=[bounce[l][_j][:]],
                            outs=[tbl[l][N_CORES * _o:N_CORES * (_o + _ln), :]],
                            replica_groups=[list(range(N_CORES))])
                else:
                    nc.sync.dma_start(out=tbl[l][0:nlocp, :],
                                      in_=bounce[l][0][:])

            # ===== round 1 =====
            produce_g(0)
            allgather(0)
            message_passing(0, b_t[0])
            # ===== round 2 =====
            produce_g(1)
            allgather(1)
            message_passing(1, b_t[1])
            # ===== round 3 =====
            produce_h2_rows()
            allgather(2)
            message_passing(2, None)
            # o[n] = sigmoid(sum_f agg3_T[f, n] * W3[f] + b3)
            osb = cpool.tile([P, n_strips], f32, tag="osb")
            for s in range(n_strips):
                ps = gp_pool.tile([P, 1], f32, space="PSUM", tag="gps1")
                nc.tensor.matmul(
                    ps[:], slab0[:, s * P:(s + 1) * P], w3_t[:],
                    start=True, stop=True)
                nc.vector.tensor_copy(out=osb[:, s:s + 1], in_=ps[:])
            nc.scalar.activation(out=osb[:], in_=osb[:],
                                 func=AF.Sigmoid, bias=b3_t[:])
            nc.sync.dma_start(
                out=out[:].rearrange("(s p) one -> p (s one)", p=P),
                in_=osb[:])

    nc.finalize()
    return nc


def make_inputs(meta, x, W1v, b1v, W2v, b2v, W3v, b3v, nloc):
    """Per-core input dicts for run_bass_kernel_spmd."""
    nlocp = meta["nlocp"]
    iota = np.tile(np.arange(meta["swf"], dtype=np.float16), (P, 1))
    maps = []
    for c in range(N_CORES):
        xl = np.zeros((nlocp, D), np.float32)
        r = x[c * nloc:(c + 1) * nloc]
        xl[:r.shape[0]] = r
        maps.append(dict(
            xT=np.ascontiguousarray(xl.T),
            W1=W1v.astype(np.float32), W2=W2v.astype(np.float32),
            W3=W3v.astype(np.float32).reshape(P, 1),
            b1=b1v.astype(np.float32).reshape(P, 1),
            b2=b2v.astype(np.float32).reshape(P, 1),
            b3=np.full((P, 1), float(b3v.reshape(-1)[0]), np.float32),
            iota=iota,
            idxd=meta["idx_wrapped"][c],
            dstpd=meta["dstp_g"][c].astype(np.float16),
            dinvTd=meta["dinvT"][c],
            dinvRd=np.tile(meta["dinv_row"][c], (P, 1)),
        ))
    return maps


def kernel(x, ei, W1, b1, W2, b2, W3, b3):
    x = np.asarray(x, dtype=np.float32)
    ei_np = np.asarray(ei)
    meta = build(ei_np, N_NODES, NLOC, batch_groups=32, sw=2)
    nc = build_program(meta)
    in_maps = make_inputs(meta, x,
                          np.asarray(W1), np.asarray(b1),
                          np.asarray(W2), np.asarray(b2),
                          np.asarray(W3), np.asarray(b3), NLOC)
    res = run_bass_kernel_spmd(nc, in_maps, list(range(N_CORES)))
    out = np.concatenate(
        [res.results[c]["out"].reshape(-1)[:NLOC] for c in range(N_CORES)])
    return out.reshape(N_NODES, 1).astype(np.float32)


# revision 29
# speedup vs baseline: 1.0477x; 1.0477x over previous
"""DiscGCN (3-layer GCN, 100k nodes / 1.6M edges) on 8 Trainium2 NeuronCores.

Strategy: nodes + incident (dst) edges sharded across the 8 cores; per layer
the transformed features (pre-scaled by dinv[src]) are AllGathered (in
int16-addressable chunks) into a replicated DRAM table, per-edge messages are
fetched with dma_gather round-robined over 4 SWDGE queues, and aggregated per
256-wide destination super-strip with one-hot matmuls accumulating in PSUM
(f32). The dst-side dinv factor is applied once per super-strip after
accumulation. Pad slots are masked by dstp=-1 (zero row in the one-hot A
matrix). Tables/messages bf16; accumulation f32.
"""
import numpy as np

import concourse.bacc as bacc
import concourse.bass as bass
import concourse.tile as tile
from concourse import mybir
from concourse.bass_utils import run_bass_kernel_spmd
from concourse.masks import make_identity

N_CORES = 8
D = 128
P = 128

N_NODES = 100000
NLOC = 12500

N_QUEUES = 4


def build(ei, n_nodes, nloc, batch_groups=64, sw=2):
    nlocp = ((nloc + P - 1) // P) * P
    n_strips = nlocp // P
    swf = sw * P
    n_super = (n_strips + sw - 1) // sw
    vtab = N_CORES * nlocp
    n_chunks = max(1, int(np.ceil(vtab / 32768 + 1e-9)))
    sl = int(np.ceil(nlocp / n_chunks / P)) * P
    while N_CORES * sl > 32767:
        n_chunks += 1
        sl = int(np.ceil(nlocp / n_chunks / P)) * P
    chunk_off = [min(j * sl, nlocp) for j in range(n_chunks + 1)]
    chunk_len = [chunk_off[j + 1] - chunk_off[j] for j in range(n_chunks)]
    coff = np.array(chunk_off[:-1])

    src = np.asarray(ei[0], dtype=np.int64)
    dst = np.asarray(ei[1], dtype=np.int64)

    deg = 1.0 + np.bincount(dst, minlength=n_nodes).astype(np.float32)
    dinv = (1.0 / np.sqrt(deg)).astype(np.float32)

    src_core = src // nloc
    src_loc = src % nloc
    src_chunk = np.minimum(src_loc // sl, n_chunks - 1)
    src_idx = (src_core * np.array(chunk_len)[src_chunk]
               + (src_loc - coff[src_chunk]))
    src_rowabs = N_CORES * coff[src_chunk] + src_idx

    core = dst // nloc
    dloc = dst % nloc
    sstrip = dloc // swf
    dstp = dloc % swf

    per_core = []
    for c in range(N_CORES):
        m = core == c
        e_idx = src_idx[m]
        e_rowabs = src_rowabs[m]
        e_ss = sstrip[m]
        e_dstp = dstp[m]
        e_chunk = src_chunk[m]
        order = np.lexsort((e_idx, e_ss, e_chunk))
        per_core.append((e_chunk[order], e_ss[order], e_idx[order],
                         e_rowabs[order], e_dstp[order]))

    run_groups = np.zeros((n_chunks, n_super), np.int64)
    run_lens = np.zeros((N_CORES, n_chunks, n_super), np.int64)
    for c in range(N_CORES):
        e_chunk, e_ss = per_core[c][0], per_core[c][1]
        key = e_chunk * n_super + e_ss
        cnt = np.bincount(key, minlength=n_chunks * n_super).reshape(
            n_chunks, n_super)
        run_lens[c] = cnt
        run_groups = np.maximum(run_groups, (cnt + P - 1) // P)

    # self groups first: groups [0, n_strips), padded to batch boundary
    self_off = ((n_strips + batch_groups - 1) // batch_groups) * batch_groups
    grid = []  # (chunk, sstrip, group_start, n_groups)
    gstart = self_off
    for ck in range(n_chunks):
        for ss in range(n_super):
            ng = int(run_groups[ck, ss])
            if ng == 0:
                continue
            grid.append((ck, ss, gstart, ng))
            gstart += ng
    n_groups_reg = gstart
    n_groups_reg_pad = ((n_groups_reg + batch_groups - 1)
                        // batch_groups) * batch_groups
    n_batches = n_groups_reg_pad // batch_groups
    n_groups_pad = n_groups_reg_pad
    nslot = n_groups_pad * P

    idx16 = np.full((N_CORES, nslot), -1, np.int16)
    slot_dstp = np.full((N_CORES, nslot), -1.0, np.float32)
    slot_row_abs = np.zeros((N_CORES, nslot), np.int64)

    group_chunk = np.zeros(n_groups_pad, np.int64)
    group_ss = np.full(n_groups_pad, -1, np.int64)
    for (ck, ss, g0, ng) in grid:
        group_chunk[g0:g0 + ng] = ck
        group_ss[g0:g0 + ng] = ss
    if grid and n_groups_reg < n_groups_reg_pad:
        group_chunk[n_groups_reg:n_groups_reg_pad] = grid[-1][0]

    base_of_chunk = N_CORES * coff
    for c in range(N_CORES):
        e_chunk, e_ss, e_idx, e_rowabs, e_dstp = per_core[c]
        pos = 0
        for (ck, ss, g0, ng) in grid:
            ln = int(run_lens[c, ck, ss])
            sl0 = g0 * P
            idx16[c, sl0:sl0 + ln] = e_idx[pos:pos + ln].astype(np.int16)
            slot_row_abs[c, sl0:sl0 + ln] = e_rowabs[pos:pos + ln]
            slot_dstp[c, sl0:sl0 + ln] = e_dstp[pos:pos + ln].astype(np.float32)
            pos += ln
        assert pos == len(e_idx)
        # pad slots of regular groups gather row 0 of their chunk (valid,
        # finite data); dstp stays -1 so the one-hot A row is all zero.
        rng = np.arange(self_off * P, n_groups_reg_pad * P)
        mpad = idx16[c, self_off * P:n_groups_reg_pad * P] < 0
        gch = group_chunk[rng // P]
        idx16[c, self_off * P:n_groups_reg_pad * P][mpad] = 0
        slot_row_abs[c, self_off * P:n_groups_reg_pad * P][mpad] = \
            base_of_chunk[gch][mpad]

    # self groups: one group per strip; slot p -> local node s*P+p.
    # Table rows are pre-scaled by dinv[src]; output is scaled by dinv[dst];
    # so a plain one-hot diagonal gives the dinv^2 self contribution.
    self_groups = []  # (batch, g_rel, strip)
    for s in range(n_strips):
        g_abs = s
        b = g_abs // batch_groups
        self_groups.append((b, g_abs - b * batch_groups, s))
        group_ss[g_abs] = s // sw
        sl0 = g_abs * P
        for c in range(N_CORES):
            pvec = np.arange(P)
            dlocv = s * P + pvec
            valid = dlocv < nloc
            dp = np.full(P, -1.0, np.float32)
            dp[valid] = ((s % sw) * P + pvec[valid]).astype(np.float32)
            slot_dstp[c, sl0:sl0 + P] = dp
            jv = np.minimum(dlocv // sl, n_chunks - 1)
            slot_row_abs[c, sl0:sl0 + P] = (
                N_CORES * coff[jv] + c * np.array(chunk_len)[jv]
                + (dlocv - coff[jv]))  # for simulation

    # gather segments: per batch, consecutive regular groups sharing a chunk,
    # <=32 groups; (batch, glo, ghi, chunk, seg_id)
    segments = []
    for b in range(n_batches):
        glo = b * batch_groups
        ghi_b = n_groups_reg_pad if b == n_batches - 1 else (b + 1) * batch_groups
        if glo < self_off:
            glo = self_off  # skip self batches
            if glo >= ghi_b:
                continue
        g = glo
        while g < ghi_b:
            ck = group_chunk[g]
            h = g
            while h < ghi_b and group_chunk[h] == ck and h - g < 16:
                h += 1
            sid = len(segments)
            segments.append((b, g - b * batch_groups, h - b * batch_groups,
                             int(ck), sid))
            g = h

    # subruns: self first, then regular (grid order); each subrun is a
    # contiguous group range within one batch targeting one super-strip.
    subruns = []  # (batch, ss, glo, ghi, first, last)
    seen_first = set()
    for (b, g_rel, s) in self_groups:
        ss = s // sw
        subruns.append([b, ss, g_rel, g_rel + 1, ss not in seen_first, False])
        seen_first.add(ss)
    for (ck, ss, g0, ng) in grid:
        g = g0
        while g < g0 + ng:
            b = g // batch_groups
            ghi = min(g0 + ng, (b + 1) * batch_groups)
            subruns.append([b, ss, g - b * batch_groups, ghi - b * batch_groups,
                            ss not in seen_first, False])
            seen_first.add(ss)
            g = ghi
    # mark the last subrun of each super-strip (epilogue point)
    last_of = {}
    for i, sr in enumerate(subruns):
        last_of[sr[1]] = i
    for i in last_of.values():
        subruns[i][5] = True
    subruns = [tuple(sr) for sr in subruns]

    bg16 = batch_groups * P // 16
    idx_wrapped = np.zeros((N_CORES, 128, n_batches * bg16), np.int16)
    for c in range(N_CORES):
        for b in range(n_batches):
            fl = idx16[c, b * batch_groups * P:(b + 1) * batch_groups * P]
            w = fl.reshape(-1, 16).T
            idx_wrapped[c, :, b * bg16:(b + 1) * bg16] = np.tile(w, (8, 1))

    dstp_g = np.zeros((N_CORES, 128, n_groups_pad), np.float32)
    for c in range(N_CORES):
        dstp_g[c] = slot_dstp[c].reshape(n_groups_pad, P).T

    # per-core dinv in slab layout [P, n_strips] and row layout [1, nlocp]
    dinvT = np.zeros((N_CORES, P, n_strips), np.float32)
    dinv_row = np.zeros((N_CORES, 1, nlocp), np.float32)
    for c in range(N_CORES):
        dl = np.zeros(nlocp, np.float32)
        r = dinv[c * nloc:(c + 1) * nloc]
        dl[:r.shape[0]] = r
        dinvT[c] = dl.reshape(n_strips, P).T
        dinv_row[c, 0] = dl

    return dict(
        nlocp=nlocp, n_strips=n_strips, n_super=n_super, sw=sw, swf=swf,
        vtab=vtab, n_chunks=n_chunks, chunk_off=chunk_off, chunk_len=chunk_len,
        n_batches=n_batches, batch_groups=batch_groups, nslot=nslot,
        n_groups_pad=n_groups_pad, grid=grid, group_ss=group_ss,
        segments=segments, subruns=subruns, self_groups=self_groups,
        idx_wrapped=idx_wrapped, dstp_g=dstp_g,
        slot_row_abs=slot_row_abs, slot_dstp=slot_dstp,
        dinv=dinv, dinvT=dinvT, dinv_row=dinv_row,
    )


def host_simulate(ei, x, Ws, bs, meta, nloc):
    """Vectorized numpy simulation of the device algorithm (layout check)."""
    nlocp, vtab, swf = meta["nlocp"], meta["vtab"], meta["swf"]
    group_ss = meta["group_ss"]
    slot_ss = np.repeat(group_ss, P)
    dinv = meta["dinv"]
    chunk_off, chunk_len = meta["chunk_off"], meta["chunk_len"]

    def to_table(h_list):
        """chunk-interleaved layout: chunk j holds rows
        [8*off_j + c*len_j + (loc-off_j)] for core c, local row loc."""
        t = np.zeros((vtab, h_list[0].shape[1]), np.float32)
        for j in range(len(chunk_len)):
            o, ln = chunk_off[j], chunk_len[j]
            for c in range(N_CORES):
                base = N_CORES * o + c * ln
                t[base:base + ln] = h_list[c][o:o + ln]
        return t

    x_loc = []
    dinv_loc = []
    for c in range(N_CORES):
        xx = np.zeros((nlocp, x.shape[1]), np.float32)
        r = x[c * nloc:(c + 1) * nloc]
        xx[:r.shape[0]] = r
        x_loc.append(xx)
        dl = np.zeros((nlocp,), np.float32)
        dl[:r.shape[0]] = dinv[c * nloc:c * nloc + r.shape[0]]
        dinv_loc.append(dl)

    h = x_loc
    outs = []
    for l, (W, b) in enumerate(zip(Ws, bs)):
        last = l == len(Ws) - 1
        if not last:
            t = to_table([dinv_loc[c][:, None] * (h[c] @ W)
                          for c in range(N_CORES)])
        else:
            t = to_table([dinv_loc[c][:, None] * h[c] for c in range(N_CORES)])
        hn = []
        for c in range(N_CORES):
            rows = meta["slot_row_abs"][c]
            msgs = t[rows]
            dd = meta["slot_dstp"][c]
            valid = (dd >= 0) & (slot_ss >= 0)
            a = np.zeros((nlocp, t.shape[1]), np.float32)
            tgt = slot_ss * swf + dd.astype(np.int64)
            np.add.at(a, tgt[valid], msgs[valid])
            a = a * dinv_loc[c][:, None]
            if not last:
                hn.append(np.maximum(a + b[None, :], 0.0))
            else:
                outs.append(1.0 / (1.0 + np.exp(-(a @ W + b)))[:nloc])
        if not last:
            h = hn
    return np.concatenate(outs, axis=0)


f32 = mybir.dt.float32
bf16 = mybir.dt.bfloat16
f16 = mybir.dt.float16
i16 = mybir.dt.int16
AF = mybir.ActivationFunctionType
ALU = mybir.AluOpType


def build_program(meta, use_cc=True, mdt=bf16):
    """meta: from build(). Returns finalized nc."""
    nlocp = meta["nlocp"]
    n_strips = meta["n_strips"]
    vtab = meta["vtab"]
    chunk_off = meta["chunk_off"]
    chunk_len = meta["chunk_len"]
    nb = meta["n_batches"]
    bg = meta["batch_groups"]
    sw = meta["sw"]
    swf = meta["swf"]
    n_super = meta["n_super"]
    assert swf <= 512

    segments = meta["segments"]          # (batch, glo, ghi, chunk, seg_id)
    self_groups = meta["self_groups"]    # (batch, g_rel, strip)
    subruns = meta["subruns"]            # (batch, ss, glo, ghi, first, last)
    bg16 = bg * P // 16

    nc = bacc.Bacc(None, target_bir_lowering=False, num_swdge_queues=N_QUEUES)

    # ---- I/O ----
    xT = nc.dram_tensor("xT", [P, nlocp], f32, kind="ExternalInput")
    W1 = nc.dram_tensor("W1", [P, D], f32, kind="ExternalInput")
    W2 = nc.dram_tensor("W2", [P, D], f32, kind="ExternalInput")
    W3 = nc.dram_tensor("W3", [P, 1], f32, kind="ExternalInput")
    b1 = nc.dram_tensor("b1", [P, 1], f32, kind="ExternalInput")
    b2 = nc.dram_tensor("b2", [P, 1], f32, kind="ExternalInput")
    b3 = nc.dram_tensor("b3", [P, 1], f32, kind="ExternalInput")
    iota = nc.dram_tensor("iota", [P, swf], f16, kind="ExternalInput")
    idxd = nc.dram_tensor("idxd", [P, nb * bg16], i16, kind="ExternalInput")
    dstpd = nc.dram_tensor("dstpd", [P, nb * bg], f16, kind="ExternalInput")
    dinvTd = nc.dram_tensor("dinvTd", [P, n_strips], f32, kind="ExternalInput")
    dinvRd = nc.dram_tensor("dinvRd", [P, nlocp], f32, kind="ExternalInput")
    out = nc.dram_tensor("out", [nlocp, 1], f32, kind="ExternalOutput")

    # ---- internal DRAM ----
    n_sl = len(chunk_len)
    bounce = [[nc.dram_tensor(f"bounce{l}_{j}", [chunk_len[j], D], mdt)
               for j in range(n_sl)] for l in range(3)]

    _co = {}

    def chunk_off_of(bt):
        return _co[bt.name]

    def bounce_rows(l, r0, r1):
        """list of (tensor, lo, hi) covering local rows [r0, r1)."""
        parts = []
        for j in range(n_sl):
            o = chunk_off[j]
            lo = max(r0, o)
            hi = min(r1, o + chunk_len[j])
            if lo < hi:
                parts.append((bounce[l][j], lo - o, hi - o))
        return parts
    for l in range(3):
        for j in range(n_sl):
            _co[bounce[l][j].name] = chunk_off[j]
    tbl = [nc.dram_tensor(f"tbl{l}", [vtab, D], mdt,
                          addr_space="Shared" if use_cc else "Local")
           for l in range(3)]

    with tile.TileContext(nc) as tc:
        with (
            tc.tile_pool(name="const", bufs=1) as cpool,
            tc.tile_pool(name="slab", bufs=1) as slab_pool,
            tc.tile_pool(name="stream", bufs=5) as spool,
            tc.tile_pool(name="abuf", bufs=4) as apool,
            tc.tile_pool(name="msg", bufs=5) as mpool,
            tc.tile_pool(name="stage", bufs=2) as stpool,
            tc.tile_pool(name="accp", bufs=4, space="PSUM") as acc_pool,
            tc.tile_pool(name="gp", bufs=2, space="PSUM") as gp_pool,
        ):
            # constants
            iota_t = cpool.tile([P, swf], f16)
            nc.sync.dma_start(out=iota_t[:], in_=iota[:])
            w_t = []
            for l, W in enumerate((W1, W2)):
                t = cpool.tile([P, D], f32, tag=f"w{l}")
                nc.sync.dma_start(out=t[:], in_=W[:])
                w_t.append(t)
            w3_t = cpool.tile([P, 1], f32)
            nc.sync.dma_start(out=w3_t[:], in_=W3[:])
            b_t = []
            for l, B in enumerate((b1, b2)):
                t = cpool.tile([P, 1], f32, tag=f"b{l}")
                nc.sync.dma_start(out=t[:], in_=B[:])
                b_t.append(t)
            b3_t = cpool.tile([P, 1], f32)
            nc.sync.dma_start(out=b3_t[:], in_=b3[:])
            dinvT_t = cpool.tile([P, n_strips], f32)
            nc.sync.dma_start(out=dinvT_t[:], in_=dinvTd[:])
            ident = cpool.tile([P, P], f32)
            make_identity(nc, ident[:])

            for _i in range(5):
                zm = mpool.tile([P, bg, D], mdt, tag="msg")
                nc.vector.memset(zm[:], 0.0)
            slab0 = slab_pool.tile([P, nlocp], f32, tag="s0")

            xt_sb = slab_pool.tile([P, nlocp], f32, tag="s0")
            nc.sync.dma_start(out=xt_sb[:], in_=xT[:])

            def produce_g(l):
                """round l in {0,1}: rows of dinv * (h @ W_l) -> bounce[l]."""
                stage = None
                for s in range(n_strips):
                    if l == 0:
                        lhsT = xt_sb[:, s * P:(s + 1) * P]
                    else:
                        lhsT = slab0[:, s * P:(s + 1) * P]
                    ps = gp_pool.tile([P, D], f32, space="PSUM", tag="gps")
                    nc.tensor.matmul(ps[:], lhsT, w_t[l][:], start=True, stop=True)
                    k = s % 4
                    if k == 0:
                        stage = stpool.tile([P, 4, D], mdt, tag="gstage")
                    nc.scalar.activation(out=stage[:, k, :], in_=ps[:],
                                         func=AF.Copy,
                                         scale=dinvT_t[:, s:s + 1])
                    if k == 3 or s == n_strips - 1:
                        s0 = s - k
                        for (bt, lo, hi) in bounce_rows(l, s0 * P, (s + 1) * P):
                            dv = bt[lo:hi, :].rearrange("(g p) f -> p g f", p=P)
                            g0 = (chunk_off_of(bt) + lo - s0 * P) // P
                            nc.sync.dma_start(
                                out=dv, in_=stage[:, g0:g0 + (hi - lo) // P, :])

            def produce_h2_rows():
                """transpose slab ([f, n]) into bounce[2] rows, x dinv."""
                stage = None
                for s in range(n_strips):
                    ps = gp_pool.tile([P, P], f32, space="PSUM", tag="gps")
                    nc.tensor.transpose(
                        out=ps[:], in_=slab0[:, s * P:(s + 1) * P],
                        identity=ident[:])
                    k = s % 4
                    if k == 0:
                        stage = stpool.tile([P, 4, D], mdt, tag="gstage")
                    nc.scalar.activation(out=stage[:, k, :], in_=ps[:],
                                         func=AF.Copy,
                                         scale=dinvT_t[:, s:s + 1])
                    if k == 3 or s == n_strips - 1:
                        s0 = s - k
                        for (bt, lo, hi) in bounce_rows(2, s0 * P, (s + 1) * P):
                            dv = bt[lo:hi, :].rearrange("(g p) f -> p g f", p=P)
                            g0 = (chunk_off_of(bt) + lo - s0 * P) // P
                            nc.sync.dma_start(
                                out=dv, in_=stage[:, g0:g0 + (hi - lo) // P, :])

            seg_counter = [0]

            def message_passing(l, relu_bias):
                """aggregate tbl[l] messages into slab0 (T layout)."""
                t = tbl[l]
                msg_of_batch = {}
                dst_of_batch = {}

                def ensure_batch(b):
                    if b in msg_of_batch:
                        return
                    mt = mpool.tile([P, bg, D], mdt, tag="msg")
                    msg_of_batch[b] = mt
                    it = spool.tile([P, bg16], i16, tag="idx")
                    nc.sync.dma_start(out=it[:],
                                      in_=idxd[:, b * bg16:(b + 1) * bg16])
                    dt_ = spool.tile([P, bg], f16, tag="dst")
                    nc.sync.dma_start(out=dt_[:],
                                      in_=dstpd[:, b * bg:(b + 1) * bg])
                    dst_of_batch[b] = dt_
                    # emit gathers for this batch (round-robin SWDGE queues)
                    for (bb, glo, ghi, ck, sid) in segments:
                        if bb != b:
                            continue
                        n_idx = (ghi - glo) * P
                        o8 = N_CORES * chunk_off[ck]
                        l8 = N_CORES * chunk_len[ck]
                        nc.gpsimd.dma_gather(
                            mt[:, glo:ghi, :],
                            t[o8:o8 + l8, :],
                            it[:, glo * 8:ghi * 8],
                            n_idx, n_idx, D, single_packet=False,
                            queue_num=seg_counter[0] % N_QUEUES,
                        )
                        seg_counter[0] += 1
                    # self groups: DMA pre-scaled rows from bounce
                    runs = []
                    for (bb, g_rel, s) in self_groups:
                        if bb != b:
                            continue
                        if runs and runs[-1][1] == g_rel and runs[-1][3] == s:
                            runs[-1] = (runs[-1][0], g_rel + 1, runs[-1][2], s + 1)
                        else:
                            runs.append((g_rel, g_rel + 1, s, s + 1))
                    for (g0, g1, s0, s1) in runs:
                        for (bt, lo, hi) in bounce_rows(l, s0 * P, s1 * P):
                            dv = bt[lo:hi, :].rearrange("(g p) f -> p g f", p=P)
                            gg0 = g0 + (chunk_off_of(bt) + lo - s0 * P) // P
                            nc.sync.dma_start(
                                out=mt[:, gg0:gg0 + (hi - lo) // P, :], in_=dv)

                AB = 8  # groups per batched A-build
                a_of_batch = {}

                def ensure_abuilds(b):
                    if b in a_of_batch:
                        return
                    dt_ = dst_of_batch[b]
                    tiles = []
                    for w in range(bg // AB):
                        At = apool.tile([P, AB, swf], mdt, tag="A")
                        iota_b = bass.AP(iota_t[:].tensor, iota_t[:].offset,
                                         [iota_t[:].ap[0], [0, AB],
                                          iota_t[:].ap[1]])
                        nc.vector.tensor_tensor(
                            out=At[:], in0=iota_b,
                            in1=dt_[:, w * AB:(w + 1) * AB].to_broadcast(
                                [P, AB, swf]),
                            op=ALU.is_equal,
                        )
                        tiles.append(At)
                    a_of_batch[b] = tiles

                # subruns in order; accumulate in SBUF slab per super-strip
                for (b, ss, glo, ghi, first, last) in subruns:
                    ensure_batch(b)
                    ensure_abuilds(b)
                    mt = msg_of_batch[b]
                    at = a_of_batch[b]
                    wid = min(swf, nlocp - ss * swf)
                    ps = acc_pool.tile([P, swf], f32, space="PSUM", tag="acc")
                    for g in range(glo, ghi):
                        nc.tensor.matmul(
                            ps[:], mt[:, g, :], at[g // AB][:, g % AB, :],
                            start=(g == glo), stop=(g == ghi - 1),
                        )
                    dstv = slab0[:, ss * swf:ss * swf + wid]
                    if first:
                        nc.vector.tensor_copy(out=dstv, in_=ps[:, :wid])
                    else:
                        nc.vector.tensor_add(out=dstv, in0=dstv, in1=ps[:, :wid])
                    if last:
                        # dst-side dinv scale, then ReLU+bias (layers 1-2)
                        dbc = spool.tile([P, swf], f32, tag="dbc")
                        nc.sync.dma_start(
                            out=dbc[:, :wid],
                            in_=dinvRd[:, ss * swf:ss * swf + wid])
                        nc.vector.tensor_tensor(out=dstv, in0=dstv,
                                                in1=dbc[:, :wid],
                                                op=ALU.mult)
                        if relu_bias is not None:
                            nc.scalar.activation(out=dstv, in_=dstv,
                                                 func=AF.Relu,
                                                 bias=relu_bias[:])

            def allgather(l):
                if use_cc:
                    for _j in range(len(chunk_len)):
                        _o, _ln = chunk_off[_j], chunk_len[_j]
                        nc.gpsimd.collective_compute(
                            "AllGather", ALU.bypass,
                            ins=[bounce[l][_j][:]],
                            outs=[tbl[l][N_CORES * _o:N_CORES * (_o + _ln), :]],
                            replica_groups=[list(range(N_CORES))])
                else:
                    nc.sync.dma_start(out=tbl[l][0:nlocp, :],
                                      in_=bounce[l][0][:])

            # ===== round 1 =====
            produce_g(0)
            allgather(0)
            message_passing(0, b_t[0])
            # ===== round 2 =====
            produce_g(1)
            allgather(1)
            message_passing(1, b_t[1])
            # ===== round 3 =====
            produce_h2_rows()
            allgather(2)
            message_passing(2, None)
            # o[n] = sigmoid(sum_f agg3_T[f, n] * W3[f] + b3)
            osb = cpool.tile([P, n_strips], f32, tag="osb")
            for s in range(n_strips):
                ps = gp_pool.tile([P, 1], f32, space="PSUM", tag="gps1")
                nc.tensor.matmul(
                    ps[:], slab0[:, s * P:(s + 1) * P], w3_t[:],
                    start=True, stop=True)
                nc.vector.tensor_copy(out=osb[:, s:s + 1], in_=ps[:])
            nc.scalar.activation(out=osb[:], in_=osb[:],
                                 func=AF.Sigmoid, bias=b3_t[:])
            nc.sync.dma_start(
                out=out[:].rearrange("(s p) one -> p (s one)", p=P),
                in_=osb[:])

    nc.finalize()
    return nc


def make_inputs(meta, x, W1v, b1v, W2v, b2v, W3v, b3v, nloc):
    """Per-core input dicts for run_bass_kernel_spmd."""
    nlocp = meta["nlocp"]
    iota = np.tile(np.arange(meta["swf"], dtype=np.float16), (P, 1))
    maps = []
    for c in range(N_CORES):
        xl = np.zeros((nlocp, D), np.float32)
        r = x[c * nloc:(c + 1) * nloc]
        xl[:r.shape[0]] = r
        maps.append(dict(
            xT=np.ascontiguousarray(xl.T),
            W1=W1v.astype(np.float32), W2=W2v.astype(np.float32),
            W3=W3v.astype(np.float32).reshape(P, 1),
            b1=b1v.astype(np.float32).reshape(P, 1),
            b2=b2v.astype(np.float32).reshape(P, 1),
            b3=np.full((P, 1), float(b3v.reshape(-1)[0]), np.float32),
            iota=iota,
            idxd=meta["idx_wrapped"][c],
            dstpd=meta["dstp_g"][c].astype(np.float16),
            dinvTd=meta["dinvT"][c],
            dinvRd=np.tile(meta["dinv_row"][c], (P, 1)),
        ))
    return maps


def kernel(x, ei, W1, b1, W2, b2, W3, b3):
    x = np.asarray(x, dtype=np.float32)
    ei_np = np.asarray(ei)
    meta = build(ei_np, N_NODES, NLOC, batch_groups=64, sw=2)
    nc = build_program(meta)
    in_maps = make_inputs(meta, x,
                          np.asarray(W1), np.asarray(b1),
                          np.asarray(W2), np.asarray(b2),
                          np.asarray(W3), np.asarray(b3), NLOC)
    res = run_bass_kernel_spmd(nc, in_maps, list(range(N_CORES)))
    out = np.concatenate(
        [res.results[c]["out"].reshape(-1)[:NLOC] for c in range(N_CORES)])
    return out.reshape(N_NODES, 1).astype(np.float32)
